# revision 8
# baseline (speedup 1.0000x reference)
"""Discounted cumulative return (reverse-time linear recurrence) on 8 TRN2 cores.

    c_t = r_t + gamma * (1 - terminal_t) * c_{t+1},  c_T = 0

Strategy: in reversed-time (scan) order, split the T=16.7M sequence into
8 cores x 128 partitions = 1024 rows of F=16384 elements. Every row is
scanned independently with the DVE tensor_tensor_scan instruction
(state = a*state + b along the free dim). Each row seeds its scan with an
H=2048-element halo (the tail of the neighboring row): the boundary
dependence decays as gamma^k and is cut exactly to zero by any terminal
in the halo (a=0), so per-row results match a full sequential f32 scan to
~1e-8 absolute without any cross-row or cross-core carry exchange.

The host-side shard step lays the data out in scan order (time reversed)
while building the per-core [128, H+F] tiles, so the device program is
pure forward-stride; unshard flips it back during the gather. The only
data duplication is the halo (~12% of input bytes).
"""
import sys

sys.path.insert(0, "/opt/trn_rl_repo")
from contextlib import ExitStack

import numpy as np

import concourse.bass as bass  # noqa: F401  (engine namespaces live on nc)
import concourse.tile as tile
from concourse import bacc, mybir
from concourse.bass_utils import run_bass_kernel_spmd

T = 16777216
M = 8                 # cores
L = T // M            # 2097152 elements per core
P = 128               # partitions
F = L // P            # 16384 elements per row
H = 2048              # halo elements per row
R = F + H             # loaded row length
S = 2048              # main stripe width (F % S == 0)
GAMMA = 0.99


def build_nc(p=P, f=F, h=H, s=S, gamma=GAMMA):
    r = f + h
    nc = bacc.Bacc("TRN2", debug=False, num_devices=M)
    term_in = nc.dram_tensor("terminal", [p, r], mybir.dt.uint8, kind="ExternalInput")
    rew_in = nc.dram_tensor("reward", [p, r], mybir.dt.float32, kind="ExternalInput")
    y_out = nc.dram_tensor("y", [p, f], mybir.dt.float32, kind="ExternalOutput")

    with tile.TileContext(nc) as tc, ExitStack() as ctx:
        bpool = ctx.enter_context(tc.tile_pool(name="b", bufs=6))
        apool = ctx.enter_context(tc.tile_pool(name="a", bufs=3))
        tpool = ctx.enter_context(tc.tile_pool(name="t", bufs=3))

        # columns are already in scan (reversed-time) order: halo stripe
        # first, then the main region; scan state chains via `initial`.
        stripes = [(0, h)] + [(h + k * s, s) for k in range(f // s)]
        prev_y = None
        for c0, w in stripes:
            tt = tpool.tile([p, w], mybir.dt.uint8, tag="t")
            nc.scalar.dma_start(tt[:], term_in[:, c0 : c0 + w])
            tb = bpool.tile([p, w], mybir.dt.float32, tag="b")
            nc.sync.dma_start(tb[:], rew_in[:, c0 : c0 + w])
            ta = apool.tile([p, w], mybir.dt.float32, tag="a")
            # a = gamma * (1 - terminal) = -gamma*t + gamma
            nc.scalar.activation(
                ta[:], tt[:], mybir.ActivationFunctionType.Copy,
                bias=gamma, scale=-gamma,
            )
            init = 0.0 if prev_y is None else prev_y[:, -1:]
            # in-place scan over the reward tile
            nc.vector.tensor_tensor_scan(
                tb[:], ta[:], tb[:], init,
                op0=mybir.AluOpType.mult, op1=mybir.AluOpType.add,
            )
            if c0 >= h:
                nc.scalar.dma_start(y_out[:, c0 - h : c0 - h + w], tb[:])
            prev_y = tb
    nc.finalize()
    return nc


def shard_inputs(terminal, reward, t=T, m=M, p=P, f=F, h=H):
    """Per-core [p, h+f] tiles; rows and columns in scan order."""
    l = p * f
    r = f + h
    term_pad = np.concatenate(
        [np.asarray(terminal).astype(np.uint8), np.ones(h, np.uint8)])
    rew_pad = np.concatenate(
        [np.asarray(reward).astype(np.float32), np.zeros(h, np.float32)])
    tw = np.lib.stride_tricks.sliding_window_view(term_pad, r)
    rw = np.lib.stride_tricks.sliding_window_view(rew_pad, r)
    in_maps = []
    for mm in range(m):
        base = t - (mm + 1) * l
        rows = base + (p - 1 - np.arange(p)) * f
        in_maps.append({
            "terminal": np.ascontiguousarray(tw[rows][:, ::-1]),
            "reward": np.ascontiguousarray(rw[rows][:, ::-1]),
        })
    return in_maps


def unshard_output(results, t=T, m=M, p=P, f=F):
    l = p * f
    full = np.empty(t, np.float32)
    for mm in range(m):
        y = np.asarray(results[mm]["y"])
        base = t - (mm + 1) * l
        full[base : base + l] = y.reshape(l)[::-1]
    return full


_NC = None


def kernel(terminal, reward):
    global _NC
    if _NC is None:
        _NC = build_nc()
    in_maps = shard_inputs(terminal, reward)
    res = run_bass_kernel_spmd(_NC, in_maps, list(range(M)))
    return unshard_output(res.results)
